# revision 1
# baseline (speedup 1.0000x reference)
"""Trainium2 Bass kernel for causal self-attention with RoPE.

Shapes: x (2, 2048, 2048), 16 heads x 128 head_dim.
Sharding: 8 cores = 2 batch x 4 head-groups (4 heads per core).
Each core computes q/k/v projections for its heads, RoPE, causal-masked
softmax attention, and a partial output projection (its head columns of
wo); the host sums the 4 partials per batch element.

Layout strategy (per core):
  - q,k built in transposed layout (head_dim on partitions, t free) so
    RoPE and the score matmuls need no on-device transposes.  The host
    permutes wq/wk columns so RoPE's even/odd pairs become the two
    partition halves, and pre-scales wq by 1/sqrt(head_dim).
  - scores computed as s^T (keys x q) per 256-query group; softmax skips
    the max-subtraction (scores are O(1) by construction); row sums via
    an ones-vector matmul; normalization folded into the PSUM eviction.
  - v computed directly in (t x e) layout by using x^T as the stationary
    operand, so the p@v matmul needs no transposes either.
  - fully-masked key blocks are skipped (host inspects the mask), which
    halves the attention work for the causal mask; mask tiles are added
    only where a block is partially masked.
  - matmuls run in float32r (tf32) which streams at full rate for moving
    dims >= 256.
"""

import sys

if "/opt/trn_rl_repo" not in sys.path:
    sys.path.insert(0, "/opt/trn_rl_repo")

import numpy as np

import concourse.bacc as bacc
import concourse.mybir as mybir
import concourse.tile as tile
from concourse.bass_utils import run_bass_kernel_spmd

B, T, D, NH, HD = 2, 2048, 2048, 16, 128
HPC = 4              # heads per core
PAIR = 256           # queries per group
NPAIR = T // PAIR    # 8
NCHUNK = T // HD     # 16 key chunks of 128
NSLICE = T // PAIR   # 8 t-slices for projections
F32R = mybir.dt.float32r
F32 = mybir.dt.float32


def _mask_structure(mask):
    """Classify each (query-group, key-chunk) block of the additive mask.

    Returns (statuses, maskt) where statuses[j] is a list of
    (chunk, mask_tile_index_or_minus1) for blocks that must be computed,
    and maskt is the packed (128, nmask, 256) array of transposed mask
    tiles for partially-masked blocks.
    """
    statuses = []
    masked = []
    for j in range(NPAIR):
        q = slice(j * PAIR, (j + 1) * PAIR)
        lst = []
        for c in range(NCHUNK):
            k = slice(c * HD, (c + 1) * HD)
            sub = mask[q, k]
            if np.all(sub <= -1e8):
                continue
            if np.all(sub == 0.0):
                lst.append((c, -1))
            else:
                masked.append((j, c))
                lst.append((c, len(masked) - 1))
        assert lst, f"query group {j} has every key block masked"
        statuses.append(lst)
    nmask = max(1, len(masked))
    maskt = np.zeros((HD, nmask, PAIR), np.float32)
    for i, (j, c) in enumerate(masked):
        q = slice(j * PAIR, (j + 1) * PAIR)
        k = slice(c * HD, (c + 1) * HD)
        maskt[:, i, :] = mask[q, k].T
    return statuses, maskt


def _build_program(statuses, nmask):
    nc = bacc.Bacc(None, target_bir_lowering=False)

    xt_d = nc.dram_tensor("xt", [D, T], F32R, kind="ExternalInput")
    wq_d = nc.dram_tensor("wqt", [D, HPC * HD], F32R, kind="ExternalInput")
    wk_d = nc.dram_tensor("wkt", [D, HPC * HD], F32R, kind="ExternalInput")
    wv_d = nc.dram_tensor("wvt", [D, HPC * HD], F32R, kind="ExternalInput")
    wo_d = nc.dram_tensor("wot", [HPC * HD, D], F32R, kind="ExternalInput")
    cs_d = nc.dram_tensor("cs", [HD, 2, T], F32, kind="ExternalInput")
    mk_d = nc.dram_tensor("maskt", [HD, nmask, PAIR], F32, kind="ExternalInput")
    ones_d = nc.dram_tensor("ones_col", [HD, 1], F32R, kind="ExternalInput")
    onesr_d = nc.dram_tensor("ones_row", [1, HD], F32R, kind="ExternalInput")
    out_d = nc.dram_tensor("out", [T, D], F32, kind="ExternalOutput")

    xt_ap = xt_d.ap().rearrange("(k p) t -> p k t", p=HD)
    EXP = mybir.ActivationFunctionType.Exp

    with tile.TileContext(nc) as tc:
        with tc.tile_pool(name="const", bufs=1) as constp, \
             tc.tile_pool(name="qkp", bufs=1) as qkp:
            ones_sb = constp.tile([HD, 1], F32R)
            onesr_sb = constp.tile([1, HD], F32R)
            nc.sync.dma_start(ones_sb[:], ones_d[:])
            nc.sync.dma_start(onesr_sb[:], onesr_d[:])
            # q heads at [:, h, :], k heads at [:, 4+h, :]
            qk_sb = qkp.tile([HD, 2 * HPC, T], F32R)

            # ---- q/k projection passes (+ fused RoPE) ----
            with tc.tile_pool(name="csp", bufs=1) as csp:
                cs_sb = csp.tile([HD, 2, T], F32)
                nc.sync.dma_start(cs_sb[:], cs_d[:])
                for wsel, w_d in ((0, wq_d), (1, wk_d)):
                    with tc.tile_pool(name="wp", bufs=1) as wp, \
                         tc.tile_pool(name="xtp", bufs=2) as xtp, \
                         tc.tile_pool(name="ropep", bufs=2) as ropep, \
                         tc.tile_pool(name="pps", bufs=4, space="PSUM") as pps:
                        w_sb = wp.tile([HD, NCHUNK, HPC * HD], F32R, name=f"w{wsel}")
                        nc.sync.dma_start(
                            w_sb[:],
                            w_d.ap().rearrange("(k p) e -> p k e", p=HD),
                        )
                        for ns in range(NSLICE):
                            tsl = slice(ns * PAIR, (ns + 1) * PAIR)
                            xt = xtp.tile([HD, NCHUNK, PAIR], F32R, tag="xt")
                            nc.sync.dma_start(xt[:], xt_ap[:, :, tsl])
                            for h in range(HPC):
                                ps = pps.tile([HD, PAIR], F32, tag="ps")
                                hs = slice(h * HD, (h + 1) * HD)
                                for k in range(NCHUNK):
                                    nc.tensor.matmul(
                                        ps[:],
                                        w_sb[:, k, hs],
                                        xt[:, k, :],
                                        start=(k == 0),
                                        stop=(k == NCHUNK - 1),
                                    )
                                # RoPE: dst = raw*C + swap(raw)*S
                                dst = qk_sb[:, wsel * HPC + h, tsl]
                                sw = ropep.tile([HD, PAIR], F32, tag="sw")
                                nc.vector.tensor_copy(sw[0:64, :], ps[64:128, :])
                                nc.vector.tensor_copy(sw[64:128, :], ps[0:64, :])
                                tb = ropep.tile([HD, PAIR], F32R, tag="tb")
                                nc.vector.tensor_mul(dst, ps[:], cs_sb[:, 0, tsl])
                                nc.vector.tensor_mul(tb[:], sw[:], cs_sb[:, 1, tsl])
                                nc.vector.tensor_add(dst, dst, tb[:])

            # ---- v projection (normal layout, x^T stationary) ----
            with tc.tile_pool(name="vap", bufs=1) as vap:
                v_all = vap.tile([HD, NCHUNK, HPC * HD], F32R)
                with tc.tile_pool(name="wvp", bufs=1) as wvp, \
                     tc.tile_pool(name="xtp2", bufs=2) as xtp2, \
                     tc.tile_pool(name="vps", bufs=4, space="PSUM") as vps:
                    wv_sb = wvp.tile([HD, NCHUNK, HPC * HD], F32R)
                    nc.sync.dma_start(
                        wv_sb[:],
                        wv_d.ap().rearrange("(k p) e -> p k e", p=HD),
                    )
                    for ns in range(NSLICE):
                        tsl = slice(ns * PAIR, (ns + 1) * PAIR)
                        xt = xtp2.tile([HD, NCHUNK, PAIR], F32R, tag="xt2")
                        nc.sync.dma_start(xt[:], xt_ap[:, :, tsl])
                        for tc2 in range(2):
                            ps = vps.tile([HD, HPC * HD], F32, tag="vps")
                            for k in range(NCHUNK):
                                nc.tensor.matmul(
                                    ps[:],
                                    xt[:, k, tc2 * HD:(tc2 + 1) * HD],
                                    wv_sb[:, k, :],
                                    start=(k == 0),
                                    stop=(k == NCHUNK - 1),
                                )
                            nc.scalar.copy(v_all[:, ns * 2 + tc2, :], ps[:])

                # ---- attention ----
                with tc.tile_pool(name="ctxp", bufs=1) as ctxp:
                    ctx_sb = ctxp.tile([HD, HPC, T], F32R)
                    with tc.tile_pool(name="ptp", bufs=2) as ptp, \
                         tc.tile_pool(name="mkp", bufs=4) as mkp, \
                         tc.tile_pool(name="lrp", bufs=2) as lrp, \
                         tc.tile_pool(name="rbp", bufs=2) as rbp, \
                         tc.tile_pool(name="sps", bufs=3, space="PSUM") as sps, \
                         tc.tile_pool(name="ops", bufs=2, space="PSUM") as ops, \
                         tc.tile_pool(name="lps", bufs=2, space="PSUM") as lps:

                        def finalize(fin):
                            h, qsl, o_ps, l_ps = fin
                            lr = lrp.tile([1, PAIR], F32R, tag="lr")
                            with nc.allow_low_precision(reason="softmax recip"):
                                nc.vector.reciprocal(lr[:], l_ps[:])
                            rb_ps = sps.tile([HD, PAIR], F32, tag="s")
                            nc.tensor.matmul(
                                rb_ps[:], onesr_sb[:], lr[:], start=True, stop=True
                            )
                            rb_sb = rbp.tile([HD, PAIR], F32, tag="rb")
                            nc.scalar.copy(rb_sb[:], rb_ps[:])
                            nc.vector.tensor_mul(
                                ctx_sb[:, h, qsl], o_ps[:], rb_sb[:]
                            )

                        pending = None
                        for h in range(HPC):
                            for j in range(NPAIR):
                                qsl = slice(j * PAIR, (j + 1) * PAIR)
                                chunks = statuses[j]
                                n = len(chunks)
                                o_ps = ops.tile([HD, PAIR], F32, tag="o")
                                l_ps = lps.tile([1, PAIR], F32, tag="l")
                                pt = ptp.tile([HD, NCHUNK, PAIR], F32R, tag="pt")
                                for i, (c, mi) in enumerate(chunks):
                                    ksl = slice(c * HD, (c + 1) * HD)
                                    s_ps = sps.tile([HD, PAIR], F32, tag="s")
                                    nc.tensor.matmul(
                                        s_ps[:],
                                        qk_sb[:, HPC + h, ksl],
                                        qk_sb[:, h, qsl],
                                        start=True,
                                        stop=True,
                                    )
                                    if mi >= 0:
                                        mt = mkp.tile([HD, PAIR], F32, tag="mk")
                                        nc.sync.dma_start(mt[:], mk_d[:, mi, :])
                                        nc.vector.tensor_add(s_ps[:], s_ps[:], mt[:])
                                    nc.scalar.activation(pt[:, i, :], s_ps[:], EXP)
                                    nc.tensor.matmul(
                                        o_ps[:],
                                        v_all[:, c, h * HD:(h + 1) * HD],
                                        pt[:, i, :],
                                        start=(i == 0),
                                        stop=(i == n - 1),
                                    )
                                    nc.tensor.matmul(
                                        l_ps[:],
                                        ones_sb[:],
                                        pt[:, i, :],
                                        start=(i == 0),
                                        stop=(i == n - 1),
                                    )
                                    if i == 2 and pending is not None:
                                        finalize(pending)
                                        pending = None
                                if pending is not None:
                                    finalize(pending)
                                pending = (h, qsl, o_ps, l_ps)
                        finalize(pending)

                    # ---- output projection ----
                    with tc.tile_pool(name="wop", bufs=1) as wop, \
                         tc.tile_pool(name="evp", bufs=4) as evp, \
                         tc.tile_pool(name="wops", bufs=4, space="PSUM") as wops:
                        wo_sb = wop.tile([HD, HPC, D], F32R)
                        nc.sync.dma_start(
                            wo_sb[:],
                            wo_d.ap().rearrange("(h p) e -> p h e", p=HD),
                        )
                        for tck in range(NCHUNK):
                            tsl = slice(tck * HD, (tck + 1) * HD)
                            for es in range(4):
                                esl = slice(es * 512, (es + 1) * 512)
                                ps = wops.tile([HD, 512], F32, tag="wo")
                                for h in range(HPC):
                                    nc.tensor.matmul(
                                        ps[:],
                                        ctx_sb[:, h, tsl],
                                        wo_sb[:, h, esl],
                                        start=(h == 0),
                                        stop=(h == HPC - 1),
                                    )
                                ev = evp.tile([HD, 512], F32, tag="ev")
                                nc.scalar.copy(ev[:], ps[:])
                                nc.sync.dma_start(out_d[tsl, esl], ev[:])
    nc.compile()
    return nc


_PERM = np.concatenate(
    [np.concatenate([np.arange(0, HD, 2), np.arange(1, HD, 2)]) + h * HD
     for h in range(HPC)]
)


def prepare(x, freqs, mask, wq, wk, wv, wo):
    """Host-side sharding/prep. Returns (nc, in_maps)."""
    x = np.asarray(x, np.float32)
    freqs = np.asarray(freqs, np.float32)
    mask = np.asarray(mask, np.float32)
    wq, wk, wv, wo = (np.asarray(w, np.float32) for w in (wq, wk, wv, wo))

    statuses, maskt = _mask_structure(mask)
    nc = _build_program(statuses, maskt.shape[1])

    scale = np.float32(1.0 / np.sqrt(HD))
    cos = np.ascontiguousarray(freqs[:, :, 0].T)  # (64, T)
    sin = np.ascontiguousarray(freqs[:, :, 1].T)
    cs = np.empty((HD, 2, T), np.float32)
    cs[0:64, 0, :] = cos
    cs[64:128, 0, :] = cos
    cs[0:64, 1, :] = -sin
    cs[64:128, 1, :] = sin

    ones_col = np.ones((HD, 1), np.float32)
    ones_row = np.ones((1, HD), np.float32)
    xt = [np.ascontiguousarray(x[b].T) for b in range(B)]

    in_maps = []
    for core in range(8):
        b, g = core // 4, core % 4
        cols = slice(g * HPC * HD, (g + 1) * HPC * HD)
        in_maps.append({
            "xt": xt[b],
            "wqt": np.ascontiguousarray((wq.T[:, cols] * scale)[:, _PERM]),
            "wkt": np.ascontiguousarray(wk.T[:, cols][:, _PERM]),
            "wvt": np.ascontiguousarray(wv.T[:, cols]),
            "wot": np.ascontiguousarray(wo.T[cols, :]),
            "cs": cs,
            "maskt": maskt,
            "ones_col": ones_col,
            "ones_row": ones_row,
        })
    return nc, in_maps


def run(x, freqs, mask, wq, wk, wv, wo, **spmd_kwargs):
    nc, in_maps = prepare(x, freqs, mask, wq, wk, wv, wo)
    res = run_bass_kernel_spmd(nc, in_maps, list(range(8)), **spmd_kwargs)
    parts = [res.results[c]["out"] for c in range(8)]
    out = np.stack([
        parts[b * 4] + parts[b * 4 + 1] + parts[b * 4 + 2] + parts[b * 4 + 3]
        for b in range(B)
    ]).astype(np.float32)
    return out, res


def kernel(x, freqs, mask, wq, wk, wv, wo):
    out, _ = run(x, freqs, mask, wq, wk, wv, wo)
    return out


# revision 3
# speedup vs baseline: 1.0505x; 1.0505x over previous
"""Trainium2 Bass kernel for causal self-attention with RoPE.

Shapes: x (2, 2048, 2048), 16 heads x 128 head_dim.
Sharding: 8 cores = 2 batch x 4 head-groups (4 heads per core).
Each core computes q/k/v projections for its heads, RoPE, causal-masked
softmax attention, and a partial output projection (its head columns of
wo); the host sums the 4 partials per batch element.

Layout strategy (per core):
  - q,k built in transposed layout (head_dim on partitions, t free) so
    RoPE and the score matmuls need no on-device transposes.  The host
    permutes wq/wk columns so RoPE's even/odd pairs become the two
    partition halves, and pre-scales wq by 1/sqrt(head_dim).
  - scores computed as s^T (keys x q) per 256-query group; softmax skips
    the max-subtraction (scores are O(1) by construction); row sums via
    an ones-vector matmul; normalization folded into the PSUM eviction.
  - v computed directly in (t x e) layout by using x^T as the stationary
    operand, so the p@v matmul needs no transposes either.
  - fully-masked key blocks are skipped (host inspects the mask), which
    halves the attention work for the causal mask; mask tiles are added
    only where a block is partially masked.
  - matmuls run in float32r (tf32) which streams at full rate for moving
    dims >= 256.
"""

import sys
from contextlib import ExitStack

if "/opt/trn_rl_repo" not in sys.path:
    sys.path.insert(0, "/opt/trn_rl_repo")

import numpy as np

import concourse.bacc as bacc
import concourse.mybir as mybir
import concourse.tile as tile
from concourse.bass_utils import run_bass_kernel_spmd

B, T, D, NH, HD = 2, 2048, 2048, 16, 128
HPC = 4              # heads per core
PAIR = 256           # queries per group
NPAIR = T // PAIR    # 8
NCHUNK = T // HD     # 16 key chunks of 128
NSLICE = T // PAIR   # 8 t-slices for projections
F32R = mybir.dt.float32r
F32 = mybir.dt.float32


def _mask_structure(mask):
    """Classify each (query-group, key-chunk) block of the additive mask.

    Returns (statuses, maskt) where statuses[j] is a list of
    (chunk, mask_tile_index_or_minus1) for blocks that must be computed,
    and maskt is the packed (128, nmask, 256) array of transposed mask
    tiles for partially-masked blocks.
    """
    statuses = []
    masked = []
    for j in range(NPAIR):
        q = slice(j * PAIR, (j + 1) * PAIR)
        lst = []
        for c in range(NCHUNK):
            k = slice(c * HD, (c + 1) * HD)
            sub = mask[q, k]
            if np.all(sub <= -1e8):
                continue
            if np.all(sub == 0.0):
                lst.append((c, -1))
            else:
                masked.append((j, c))
                lst.append((c, len(masked) - 1))
        assert lst, f"query group {j} has every key block masked"
        statuses.append(lst)
    nmask = max(1, len(masked))
    maskt = np.zeros((HD, nmask, PAIR), np.float32)
    for i, (j, c) in enumerate(masked):
        q = slice(j * PAIR, (j + 1) * PAIR)
        k = slice(c * HD, (c + 1) * HD)
        maskt[:, i, :] = mask[q, k].T
    return statuses, maskt


def _build_program(statuses, nmask):
    nc = bacc.Bacc(None, target_bir_lowering=False)

    xt_d = nc.dram_tensor("xt", [D, T], F32R, kind="ExternalInput")
    wq_d = nc.dram_tensor("wqt", [D, HPC * HD], F32R, kind="ExternalInput")
    wk_d = nc.dram_tensor("wkt", [D, HPC * HD], F32R, kind="ExternalInput")
    wv_d = nc.dram_tensor("wvt", [D, HPC * HD], F32R, kind="ExternalInput")
    wo_d = nc.dram_tensor("wot", [HPC * HD, D], F32R, kind="ExternalInput")
    cs_d = nc.dram_tensor("cs", [HD, 2, T], F32, kind="ExternalInput")
    mk_d = nc.dram_tensor("maskt", [HD, nmask, PAIR], F32, kind="ExternalInput")
    ones_d = nc.dram_tensor("ones_col", [HD, 1], F32R, kind="ExternalInput")
    onesr_d = nc.dram_tensor("ones_row", [1, HD], F32R, kind="ExternalInput")
    out_d = nc.dram_tensor("out", [T, D], F32, kind="ExternalOutput")

    xt_ap = xt_d.ap().rearrange("(k p) t -> p k t", p=HD)
    wq_ap = wq_d.ap().rearrange("(k p) e -> p k e", p=HD)
    wk_ap = wk_d.ap().rearrange("(k p) e -> p k e", p=HD)
    wv_ap = wv_d.ap().rearrange("(k p) e -> p k e", p=HD)
    wo_ap = wo_d.ap().rearrange("(h p) e -> p h e", p=HD)
    EXP = mybir.ActivationFunctionType.Exp

    with tile.TileContext(nc) as tc, ExitStack() as top:
        constp = top.enter_context(tc.tile_pool(name="const", bufs=1))
        ones_sb = constp.tile([HD, 1], F32R)
        onesr_sb = constp.tile([1, HD], F32R)
        nc.sync.dma_start(ones_sb[:], ones_d[:])
        nc.sync.dma_start(onesr_sb[:], onesr_d[:])

        qkp = top.enter_context(tc.tile_pool(name="qkp", bufs=1))
        # q heads at [:, h, :], k heads at [:, 4+h, :]
        qk_sb = qkp.tile([HD, 2 * HPC, T], F32R)

        # ---- combined q/k projection pass (+ fused RoPE) ----
        with ExitStack() as ph:
            wp = ph.enter_context(tc.tile_pool(name="wp", bufs=1))
            csp = ph.enter_context(tc.tile_pool(name="csp", bufs=2))
            xtp = ph.enter_context(tc.tile_pool(name="xtp", bufs=2))
            ropep = ph.enter_context(tc.tile_pool(name="ropep", bufs=2))
            pps = ph.enter_context(tc.tile_pool(name="pps", bufs=4, space="PSUM"))
            wqk_sb = wp.tile([HD, 2, NCHUNK, HPC * HD], F32R)
            # split weight loads per k-chunk so the first matmuls start early
            for k in range(NCHUNK):
                nc.sync.dma_start(wqk_sb[:, 0, k, :], wq_ap[:, k, :])
                nc.sync.dma_start(wqk_sb[:, 1, k, :], wk_ap[:, k, :])
            for ns in range(NSLICE):
                tsl = slice(ns * PAIR, (ns + 1) * PAIR)
                xt = xtp.tile([HD, NCHUNK, PAIR], F32R, tag="xt")
                nc.sync.dma_start(xt[:], xt_ap[:, :, tsl])
                cs_sl = csp.tile([HD, 2, PAIR], F32, tag="cs")
                nc.sync.dma_start(cs_sl[:], cs_d[:, :, tsl])
                for wsel in range(2):
                    for h in range(HPC):
                        ps = pps.tile([HD, PAIR], F32, tag="ps")
                        hs = slice(h * HD, (h + 1) * HD)
                        for k in range(NCHUNK):
                            nc.tensor.matmul(
                                ps[:],
                                wqk_sb[:, wsel, k, hs],
                                xt[:, k, :],
                                start=(k == 0),
                                stop=(k == NCHUNK - 1),
                            )
                        # RoPE: dst = raw*C + swap(raw)*S, swap folded into
                        # the second multiply's partition offsets
                        dst = qk_sb[:, wsel * HPC + h, tsl]
                        tb = ropep.tile([HD, PAIR], F32R, tag="tb")
                        nc.vector.tensor_mul(dst, ps[:], cs_sl[:, 0, :])
                        nc.vector.tensor_mul(
                            tb[0:64, :], ps[64:128, :], cs_sl[0:64, 1, :]
                        )
                        nc.vector.tensor_mul(
                            tb[64:128, :], ps[0:64, :], cs_sl[64:128, 1, :]
                        )
                        nc.vector.tensor_add(dst, dst, tb[:])

        # ---- v projection (normal layout, x^T stationary) ----
        vap = top.enter_context(tc.tile_pool(name="vap", bufs=1))
        v_all = vap.tile([HD, NCHUNK, HPC * HD], F32R)
        with ExitStack() as ph:
            wvp = ph.enter_context(tc.tile_pool(name="wvp", bufs=1))
            xtp2 = ph.enter_context(tc.tile_pool(name="xtp2", bufs=2))
            vps = ph.enter_context(tc.tile_pool(name="vps", bufs=4, space="PSUM"))
            wv_sb = wvp.tile([HD, NCHUNK, HPC * HD], F32R)
            for k in range(NCHUNK):
                nc.sync.dma_start(wv_sb[:, k, :], wv_ap[:, k, :])
            for ns in range(NSLICE):
                tsl = slice(ns * PAIR, (ns + 1) * PAIR)
                xt = xtp2.tile([HD, NCHUNK, PAIR], F32R, tag="xt2")
                nc.sync.dma_start(xt[:], xt_ap[:, :, tsl])
                for tc2 in range(2):
                    ps = vps.tile([HD, HPC * HD], F32, tag="vps")
                    for k in range(NCHUNK):
                        nc.tensor.matmul(
                            ps[:],
                            xt[:, k, tc2 * HD:(tc2 + 1) * HD],
                            wv_sb[:, k, :],
                            start=(k == 0),
                            stop=(k == NCHUNK - 1),
                        )
                    nc.scalar.copy(v_all[:, ns * 2 + tc2, :], ps[:])

        # ---- attention ----
        ctxp = top.enter_context(tc.tile_pool(name="ctxp", bufs=1))
        ctx_sb = ctxp.tile([HD, HPC, T], F32R)
        with ExitStack() as ph:
            ptp = ph.enter_context(tc.tile_pool(name="ptp", bufs=2))
            mkp = ph.enter_context(tc.tile_pool(name="mkp", bufs=4))
            lrp = ph.enter_context(tc.tile_pool(name="lrp", bufs=2))
            rbp = ph.enter_context(tc.tile_pool(name="rbp", bufs=2))
            sps = ph.enter_context(tc.tile_pool(name="sps", bufs=3, space="PSUM"))
            ops = ph.enter_context(tc.tile_pool(name="ops", bufs=2, space="PSUM"))
            lps = ph.enter_context(tc.tile_pool(name="lps", bufs=2, space="PSUM"))

            def finalize(fin):
                h, qsl, o_ps, l_ps = fin
                lr = lrp.tile([1, PAIR], F32R, tag="lr")
                with nc.allow_low_precision(reason="softmax recip"):
                    nc.vector.reciprocal(lr[:], l_ps[:])
                rb_ps = sps.tile([HD, PAIR], F32, tag="s")
                nc.tensor.matmul(rb_ps[:], onesr_sb[:], lr[:], start=True, stop=True)
                rb_sb = rbp.tile([HD, PAIR], F32, tag="rb")
                nc.scalar.copy(rb_sb[:], rb_ps[:])
                nc.vector.tensor_mul(ctx_sb[:, h, qsl], o_ps[:], rb_sb[:])

            pending = None
            for j in range(NPAIR):
                qsl = slice(j * PAIR, (j + 1) * PAIR)
                chunks = statuses[j]
                n = len(chunks)
                for h in range(HPC):
                    o_ps = ops.tile([HD, PAIR], F32, tag="o")
                    l_ps = lps.tile([1, PAIR], F32, tag="l")
                    pt = ptp.tile([HD, NCHUNK, PAIR], F32R, tag="pt")
                    for i, (c, mi) in enumerate(chunks):
                        ksl = slice(c * HD, (c + 1) * HD)
                        s_ps = sps.tile([HD, PAIR], F32, tag="s")
                        nc.tensor.matmul(
                            s_ps[:],
                            qk_sb[:, HPC + h, ksl],
                            qk_sb[:, h, qsl],
                            start=True,
                            stop=True,
                        )
                        if mi >= 0:
                            mt = mkp.tile([HD, PAIR], F32, tag="mk")
                            nc.sync.dma_start(mt[:], mk_d[:, mi, :])
                            nc.vector.tensor_add(s_ps[:], s_ps[:], mt[:])
                        nc.scalar.activation(pt[:, i, :], s_ps[:], EXP)
                        if i >= 1:
                            nc.tensor.matmul(
                                o_ps[:],
                                v_all[:, chunks[i - 1][0], h * HD:(h + 1) * HD],
                                pt[:, i - 1, :],
                                start=(i == 1),
                                stop=False,
                            )
                        if i == 2 and pending is not None:
                            finalize(pending)
                            pending = None
                    nc.tensor.matmul(
                        o_ps[:],
                        v_all[:, chunks[n - 1][0], h * HD:(h + 1) * HD],
                        pt[:, n - 1, :],
                        start=(n == 1),
                        stop=True,
                    )
                    for i in range(n):
                        nc.tensor.matmul(
                            l_ps[:],
                            ones_sb[:],
                            pt[:, i, :],
                            start=(i == 0),
                            stop=(i == n - 1),
                        )
                    if pending is not None:
                        finalize(pending)
                    pending = (h, qsl, o_ps, l_ps)
            finalize(pending)

        # ---- output projection ----
        with ExitStack() as ph:
            wop = ph.enter_context(tc.tile_pool(name="wop", bufs=1))
            evp = ph.enter_context(tc.tile_pool(name="evp", bufs=4))
            wops = ph.enter_context(tc.tile_pool(name="wops", bufs=4, space="PSUM"))
            wo_sb = wop.tile([HD, HPC, D], F32R)
            for h in range(HPC):
                nc.sync.dma_start(wo_sb[:, h, :], wo_ap[:, h, :])
            for tck in range(NCHUNK):
                tsl = slice(tck * HD, (tck + 1) * HD)
                for es in range(4):
                    esl = slice(es * 512, (es + 1) * 512)
                    ps = wops.tile([HD, 512], F32, tag="wo")
                    for h in range(HPC):
                        nc.tensor.matmul(
                            ps[:],
                            ctx_sb[:, h, tsl],
                            wo_sb[:, h, esl],
                            start=(h == 0),
                            stop=(h == HPC - 1),
                        )
                    ev = evp.tile([HD, 512], F32, tag="ev")
                    nc.vector.tensor_copy(ev[:], ps[:])
                    nc.sync.dma_start(out_d[tsl, esl], ev[:])
    nc.compile()
    return nc


_PERM = np.concatenate(
    [np.concatenate([np.arange(0, HD, 2), np.arange(1, HD, 2)]) + h * HD
     for h in range(HPC)]
)


def prepare(x, freqs, mask, wq, wk, wv, wo):
    """Host-side sharding/prep. Returns (nc, in_maps)."""
    x = np.asarray(x, np.float32)
    freqs = np.asarray(freqs, np.float32)
    mask = np.asarray(mask, np.float32)
    wq, wk, wv, wo = (np.asarray(w, np.float32) for w in (wq, wk, wv, wo))

    statuses, maskt = _mask_structure(mask)
    nc = _build_program(statuses, maskt.shape[1])

    scale = np.float32(1.0 / np.sqrt(HD))
    cos = np.ascontiguousarray(freqs[:, :, 0].T)  # (64, T)
    sin = np.ascontiguousarray(freqs[:, :, 1].T)
    cs = np.empty((HD, 2, T), np.float32)
    cs[0:64, 0, :] = cos
    cs[64:128, 0, :] = cos
    cs[0:64, 1, :] = -sin
    cs[64:128, 1, :] = sin

    ones_col = np.ones((HD, 1), np.float32)
    ones_row = np.ones((1, HD), np.float32)
    xt = [np.ascontiguousarray(x[b].T) for b in range(B)]

    in_maps = []
    for core in range(8):
        b, g = core // 4, core % 4
        cols = slice(g * HPC * HD, (g + 1) * HPC * HD)
        in_maps.append({
            "xt": xt[b],
            "wqt": np.ascontiguousarray((wq.T[:, cols] * scale)[:, _PERM]),
            "wkt": np.ascontiguousarray(wk.T[:, cols][:, _PERM]),
            "wvt": np.ascontiguousarray(wv.T[:, cols]),
            "wot": np.ascontiguousarray(wo.T[cols, :]),
            "cs": cs,
            "maskt": maskt,
            "ones_col": ones_col,
            "ones_row": ones_row,
        })
    return nc, in_maps


def run(x, freqs, mask, wq, wk, wv, wo, **spmd_kwargs):
    nc, in_maps = prepare(x, freqs, mask, wq, wk, wv, wo)
    res = run_bass_kernel_spmd(nc, in_maps, list(range(8)), **spmd_kwargs)
    parts = [res.results[c]["out"] for c in range(8)]
    out = np.stack([
        parts[b * 4] + parts[b * 4 + 1] + parts[b * 4 + 2] + parts[b * 4 + 3]
        for b in range(B)
    ]).astype(np.float32)
    return out, res


def kernel(x, freqs, mask, wq, wk, wv, wo):
    out, _ = run(x, freqs, mask, wq, wk, wv, wo)
    return out


# revision 5
# speedup vs baseline: 1.0558x; 1.0051x over previous
"""Trainium2 Bass kernel for causal self-attention with RoPE.

Shapes: x (2, 2048, 2048), 16 heads x 128 head_dim.
Sharding: 8 cores = 2 batch x 4 head-groups (4 heads per core).
Each core computes q/k/v projections for its heads, RoPE, causal-masked
softmax attention, and a partial output projection (its head columns of
wo); the host sums the 4 partials per batch element.

Layout strategy (per core):
  - q,k built in transposed layout (head_dim on partitions, t free) so
    RoPE and the score matmuls need no on-device transposes.  The host
    permutes wq/wk columns so RoPE's even/odd pairs become the two
    partition halves, and pre-scales wq by 1/sqrt(head_dim).
  - scores computed as s^T (keys x q) per 256-query group; softmax skips
    the max-subtraction (scores are O(1) by construction); row sums via
    an ones-vector matmul; normalization folded into the PSUM eviction.
  - v computed directly in (t x e) layout by using x^T as the stationary
    operand, so the p@v matmul needs no transposes anywhere.
  - fully-masked key blocks are skipped (host inspects the mask), which
    halves the attention work for the causal mask; deduplicated mask
    tiles are added only where a block is partially masked.
  - matmuls run in float32r (tf32) which streams at full rate for moving
    dims >= 256.
  - weights/mask stream on the scalar-engine DMA queue, x^T/tables/output
    on the sync-engine queue, so activations never queue behind weights.
"""

import sys
from contextlib import ExitStack

if "/opt/trn_rl_repo" not in sys.path:
    sys.path.insert(0, "/opt/trn_rl_repo")

import numpy as np

import concourse.bacc as bacc
import concourse.mybir as mybir
import concourse.tile as tile
from concourse.bass_utils import run_bass_kernel_spmd

B, T, D, NH, HD = 2, 2048, 2048, 16, 128
HPC = 4              # heads per core
PAIR = 256           # queries per group
NPAIR = T // PAIR    # 8
NCHUNK = T // HD     # 16 key chunks of 128
NSLICE = T // PAIR   # 8 t-slices for projections
F32R = mybir.dt.float32r
F32 = mybir.dt.float32
MASK_PRELOAD_MAX = 24


def _mask_structure(mask):
    """Classify each (query-group, key-chunk) block of the additive mask.

    Returns (statuses, maskt): statuses[j] is a list of
    (chunk, mask_tile_index_or_minus1) for blocks that must be computed;
    maskt is the packed (128, nmask, 256) array of deduplicated
    transposed mask tiles for partially-masked blocks.
    """
    statuses = []
    tiles = {}
    tile_list = []
    for j in range(NPAIR):
        q = slice(j * PAIR, (j + 1) * PAIR)
        lst = []
        for c in range(NCHUNK):
            k = slice(c * HD, (c + 1) * HD)
            sub = mask[q, k]
            if np.all(sub <= -1e8):
                continue
            if np.all(sub == 0.0):
                lst.append((c, -1))
            else:
                key = sub.tobytes()
                mi = tiles.get(key)
                if mi is None:
                    mi = len(tile_list)
                    tiles[key] = mi
                    tile_list.append(np.ascontiguousarray(sub.T))
                lst.append((c, mi))
        assert lst, f"query group {j} has every key block masked"
        statuses.append(lst)
    nmask = max(1, len(tile_list))
    maskt = np.zeros((HD, nmask, PAIR), np.float32)
    for i, t in enumerate(tile_list):
        maskt[:, i, :] = t
    return statuses, maskt


def _build_program(statuses, nmask):
    nc = bacc.Bacc(None, target_bir_lowering=False)

    xt_d = nc.dram_tensor("xt", [D, T], F32R, kind="ExternalInput")
    wq_d = nc.dram_tensor("wqt", [D, HPC * HD], F32R, kind="ExternalInput")
    wk_d = nc.dram_tensor("wkt", [D, HPC * HD], F32R, kind="ExternalInput")
    wv_d = nc.dram_tensor("wvt", [D, HPC * HD], F32R, kind="ExternalInput")
    wo_d = nc.dram_tensor("wot", [HPC * HD, D], F32R, kind="ExternalInput")
    cs_d = nc.dram_tensor("cs", [HD, 2, T], F32, kind="ExternalInput")
    mk_d = nc.dram_tensor("maskt", [HD, nmask, PAIR], F32, kind="ExternalInput")
    ones_d = nc.dram_tensor("ones_col", [HD, 1], F32R, kind="ExternalInput")
    onesr_d = nc.dram_tensor("ones_row", [1, HD], F32R, kind="ExternalInput")
    out_d = nc.dram_tensor("out", [T, D], F32, kind="ExternalOutput")

    xt_ap = xt_d.ap().rearrange("(k p) t -> p k t", p=HD)
    wq_ap = wq_d.ap().rearrange("(k p) e -> p k e", p=HD)
    wk_ap = wk_d.ap().rearrange("(k p) e -> p k e", p=HD)
    wv_ap = wv_d.ap().rearrange("(k p) e -> p k e", p=HD)
    wo_ap = wo_d.ap().rearrange("(h p) e -> p h e", p=HD)
    EXP = mybir.ActivationFunctionType.Exp
    preload_mask = nmask <= MASK_PRELOAD_MAX

    with tile.TileContext(nc) as tc, ExitStack() as top:
        constp = top.enter_context(tc.tile_pool(name="const", bufs=1))
        ones_sb = constp.tile([HD, 1], F32R)
        onesr_sb = constp.tile([1, HD], F32R)
        nc.scalar.dma_start(ones_sb[:], ones_d[:])
        nc.scalar.dma_start(onesr_sb[:], onesr_d[:])

        qkp = top.enter_context(tc.tile_pool(name="qkp", bufs=1))
        # q heads at [:, h, :], k heads at [:, 4+h, :]
        qk_sb = qkp.tile([HD, 2 * HPC, T], F32R)

        # wv pool spans the q/k pass (prefetch) and the v pass
        with ExitStack() as vph:
            wvp = vph.enter_context(tc.tile_pool(name="wvp", side="right", bufs=1))
            wv_sb = wvp.tile([HD, NCHUNK, HPC * HD], F32R)

            # ---- combined q/k projection pass (+ fused RoPE) ----
            with ExitStack() as ph:
                wp = ph.enter_context(tc.tile_pool(name="wp", side="right", bufs=1))
                csp = ph.enter_context(tc.tile_pool(name="csp", side="right", bufs=2))
                xtp = ph.enter_context(tc.tile_pool(name="xtp", side="right", bufs=2))
                ropep = ph.enter_context(tc.tile_pool(name="ropep", side="right", bufs=2))
                pps = ph.enter_context(tc.tile_pool(name="pps", bufs=4, space="PSUM"))
                wqk_sb = wp.tile([HD, 2, NCHUNK, HPC * HD], F32R)
                # weights on the scalar queue, split per k-chunk so the
                # first matmuls start as soon as chunk 0 lands
                for k in range(NCHUNK):
                    nc.scalar.dma_start(wqk_sb[:, 0, k, :], wq_ap[:, k, :])
                for k in range(NCHUNK):
                    nc.scalar.dma_start(wqk_sb[:, 1, k, :], wk_ap[:, k, :])
                for k in range(NCHUNK):  # prefetch wv for the next pass
                    nc.scalar.dma_start(wv_sb[:, k, :], wv_ap[:, k, :])
                for ns in range(NSLICE):
                    tsl = slice(ns * PAIR, (ns + 1) * PAIR)
                    xt = xtp.tile([HD, NCHUNK, PAIR], F32R, tag="xt")
                    nc.sync.dma_start(xt[:], xt_ap[:, :, tsl])
                    cs_sl = csp.tile([HD, 2, PAIR], F32, tag="cs")
                    nc.sync.dma_start(cs_sl[:], cs_d[:, :, tsl])
                    for wsel in range(2):
                        for h in range(HPC):
                            ps = pps.tile([HD, PAIR], F32, tag="ps")
                            hs = slice(h * HD, (h + 1) * HD)
                            for k in range(NCHUNK):
                                nc.tensor.matmul(
                                    ps[:],
                                    wqk_sb[:, wsel, k, hs],
                                    xt[:, k, :],
                                    start=(k == 0),
                                    stop=(k == NCHUNK - 1),
                                )
                            # RoPE: dst = raw*C + swap(raw)*S, swap folded
                            # into the second multiply's partition offsets
                            dst = qk_sb[:, wsel * HPC + h, tsl]
                            tb = ropep.tile([HD, PAIR], F32R, tag="tb")
                            nc.vector.tensor_mul(dst, ps[:], cs_sl[:, 0, :])
                            nc.vector.tensor_mul(
                                tb[0:64, :], ps[64:128, :], cs_sl[0:64, 1, :]
                            )
                            nc.vector.tensor_mul(
                                tb[64:128, :], ps[0:64, :], cs_sl[64:128, 1, :]
                            )
                            nc.vector.tensor_add(dst, dst, tb[:])

            # ---- v projection (normal layout, x^T stationary) ----
            vap = top.enter_context(tc.tile_pool(name="vap", bufs=1))
            v_all = vap.tile([HD, NCHUNK, HPC * HD], F32R)
            with ExitStack() as ph:
                xtp2 = ph.enter_context(tc.tile_pool(name="xtp2", side="right", bufs=2))
                vps = ph.enter_context(tc.tile_pool(name="vps", bufs=4, space="PSUM"))
                for ns in range(NSLICE):
                    tsl = slice(ns * PAIR, (ns + 1) * PAIR)
                    xt = xtp2.tile([HD, NCHUNK, PAIR], F32R, tag="xt2")
                    nc.sync.dma_start(xt[:], xt_ap[:, :, tsl])
                    for tc2 in range(2):
                        ps = vps.tile([HD, HPC * HD], F32, tag="vps")
                        for k in range(NCHUNK):
                            nc.tensor.matmul(
                                ps[:],
                                xt[:, k, tc2 * HD:(tc2 + 1) * HD],
                                wv_sb[:, k, :],
                                start=(k == 0),
                                stop=(k == NCHUNK - 1),
                            )
                        nc.vector.tensor_copy(v_all[:, ns * 2 + tc2, :], ps[:])

        # ---- attention ----
        ctxp = top.enter_context(tc.tile_pool(name="ctxp", bufs=1))
        ctx_sb = ctxp.tile([HD, HPC, T], F32R)
        wop = top.enter_context(tc.tile_pool(name="wop", bufs=1))
        wo_sb = wop.tile([HD, HPC, D], F32R)
        with ExitStack() as ph:
            ptp = ph.enter_context(tc.tile_pool(name="ptp", side="right", bufs=2))
            mkp = ph.enter_context(tc.tile_pool(name="mkp", side="right", bufs=4))
            lrp = ph.enter_context(tc.tile_pool(name="lrp", side="right", bufs=2))
            rbp = ph.enter_context(tc.tile_pool(name="rbp", side="right", bufs=2))
            sps = ph.enter_context(tc.tile_pool(name="sps", bufs=3, space="PSUM"))
            ops = ph.enter_context(tc.tile_pool(name="ops", bufs=2, space="PSUM"))
            lps = ph.enter_context(tc.tile_pool(name="lps", bufs=2, space="PSUM"))

            mk_sb = None
            if preload_mask:
                mk_sb = mkp.tile([HD, nmask, PAIR], F32)
                nc.scalar.dma_start(mk_sb[:], mk_d[:])
            for h in range(HPC):  # prefetch wo for the final pass
                nc.scalar.dma_start(wo_sb[:, h, :], wo_ap[:, h, :])

            def mask_tile(mi):
                if preload_mask:
                    return mk_sb[:, mi, :]
                mt = mkp.tile([HD, PAIR], F32, tag="mk")
                nc.scalar.dma_start(mt[:], mk_d[:, mi, :])
                return mt[:]

            def finalize(fin):
                h, qsl, o_ps, l_ps = fin
                lr = lrp.tile([1, PAIR], F32R, tag="lr")
                with nc.allow_low_precision(reason="softmax recip"):
                    nc.vector.reciprocal(lr[:], l_ps[:])
                rb_ps = sps.tile([HD, 2, PAIR], F32, tag="s")
                nc.tensor.matmul(
                    rb_ps[:, 0, :], onesr_sb[:], lr[:], start=True, stop=True
                )
                rb_sb = rbp.tile([HD, PAIR], F32, tag="rb")
                nc.vector.tensor_copy(rb_sb[:], rb_ps[:, 0, :])
                nc.vector.tensor_mul(ctx_sb[:, h, qsl], o_ps[:], rb_sb[:])

            pending = None
            for j in range(NPAIR):
                qsl = slice(j * PAIR, (j + 1) * PAIR)
                chunks = statuses[j]
                n = len(chunks)
                groups = [chunks[ii:ii + 2] for ii in range(0, n, 2)]
                for h in range(HPC):
                    o_ps = ops.tile([HD, PAIR], F32, tag="o")
                    l_ps = lps.tile([1, PAIR], F32, tag="l")
                    pt = ptp.tile([HD, NCHUNK, PAIR], F32R, tag="pt")
                    prev = None  # (start_chunk_idx, group)
                    oi = 0
                    for gi, grp in enumerate(groups):
                        w = len(grp)
                        s_ps = sps.tile([HD, 2, PAIR], F32, tag="s")
                        for t, (c, mi) in enumerate(grp):
                            nc.tensor.matmul(
                                s_ps[:, t, :],
                                qk_sb[:, HPC + h, c * HD:(c + 1) * HD],
                                qk_sb[:, h, qsl],
                                start=True,
                                stop=True,
                            )
                        for t, (c, mi) in enumerate(grp):
                            if mi >= 0:
                                nc.vector.tensor_add(
                                    s_ps[:, t, :], s_ps[:, t, :], mask_tile(mi)
                                )
                        nc.scalar.activation(
                            pt[:, gi * 2:gi * 2 + w, :], s_ps[:, 0:w, :], EXP
                        )
                        if prev is not None:
                            pi, pgrp = prev
                            for t, (c, mi) in enumerate(pgrp):
                                nc.tensor.matmul(
                                    o_ps[:],
                                    v_all[:, c, h * HD:(h + 1) * HD],
                                    pt[:, pi + t, :],
                                    start=(oi == 0),
                                    stop=False,
                                )
                                oi += 1
                        if gi == 1 and pending is not None:
                            finalize(pending)
                            pending = None
                        prev = (gi * 2, grp)
                    pi, pgrp = prev
                    for t, (c, mi) in enumerate(pgrp):
                        nc.tensor.matmul(
                            o_ps[:],
                            v_all[:, c, h * HD:(h + 1) * HD],
                            pt[:, pi + t, :],
                            start=(oi == 0),
                            stop=(t == len(pgrp) - 1),
                        )
                        oi += 1
                    for i in range(n):
                        nc.tensor.matmul(
                            l_ps[:],
                            ones_sb[:],
                            pt[:, i, :],
                            start=(i == 0),
                            stop=(i == n - 1),
                        )
                    if pending is not None:
                        finalize(pending)
                    pending = (h, qsl, o_ps, l_ps)
            finalize(pending)

        # ---- output projection ----
        with ExitStack() as ph:
            evp = ph.enter_context(tc.tile_pool(name="evp", side="right", bufs=4))
            wops = ph.enter_context(tc.tile_pool(name="wops", bufs=4, space="PSUM"))
            for tck in range(NCHUNK):
                tsl = slice(tck * HD, (tck + 1) * HD)
                for es in range(4):
                    esl = slice(es * 512, (es + 1) * 512)
                    ps = wops.tile([HD, 512], F32, tag="wo")
                    for h in range(HPC):
                        nc.tensor.matmul(
                            ps[:],
                            ctx_sb[:, h, tsl],
                            wo_sb[:, h, esl],
                            start=(h == 0),
                            stop=(h == HPC - 1),
                        )
                    ev = evp.tile([HD, 512], F32, tag="ev")
                    nc.vector.tensor_copy(ev[:], ps[:])
                    nc.sync.dma_start(out_d[tsl, esl], ev[:])
    nc.compile()
    return nc


_PERM = np.concatenate(
    [np.concatenate([np.arange(0, HD, 2), np.arange(1, HD, 2)]) + h * HD
     for h in range(HPC)]
)


def prepare(x, freqs, mask, wq, wk, wv, wo):
    """Host-side sharding/prep. Returns (nc, in_maps)."""
    x = np.asarray(x, np.float32)
    freqs = np.asarray(freqs, np.float32)
    mask = np.asarray(mask, np.float32)
    wq, wk, wv, wo = (np.asarray(w, np.float32) for w in (wq, wk, wv, wo))

    statuses, maskt = _mask_structure(mask)
    nc = _build_program(statuses, maskt.shape[1])

    scale = np.float32(1.0 / np.sqrt(HD))
    cos = np.ascontiguousarray(freqs[:, :, 0].T)  # (64, T)
    sin = np.ascontiguousarray(freqs[:, :, 1].T)
    cs = np.empty((HD, 2, T), np.float32)
    cs[0:64, 0, :] = cos
    cs[64:128, 0, :] = cos
    cs[0:64, 1, :] = -sin
    cs[64:128, 1, :] = sin

    ones_col = np.ones((HD, 1), np.float32)
    ones_row = np.ones((1, HD), np.float32)
    xt = [np.ascontiguousarray(x[b].T) for b in range(B)]

    in_maps = []
    for core in range(8):
        b, g = core // 4, core % 4
        cols = slice(g * HPC * HD, (g + 1) * HPC * HD)
        in_maps.append({
            "xt": xt[b],
            "wqt": np.ascontiguousarray((wq.T[:, cols] * scale)[:, _PERM]),
            "wkt": np.ascontiguousarray(wk.T[:, cols][:, _PERM]),
            "wvt": np.ascontiguousarray(wv.T[:, cols]),
            "wot": np.ascontiguousarray(wo.T[cols, :]),
            "cs": cs,
            "maskt": maskt,
            "ones_col": ones_col,
            "ones_row": ones_row,
        })
    return nc, in_maps


def run(x, freqs, mask, wq, wk, wv, wo, **spmd_kwargs):
    nc, in_maps = prepare(x, freqs, mask, wq, wk, wv, wo)
    res = run_bass_kernel_spmd(nc, in_maps, list(range(8)), **spmd_kwargs)
    parts = [res.results[c]["out"] for c in range(8)]
    out = np.stack([
        parts[b * 4] + parts[b * 4 + 1] + parts[b * 4 + 2] + parts[b * 4 + 3]
        for b in range(B)
    ]).astype(np.float32)
    return out, res


def kernel(x, freqs, mask, wq, wk, wv, wo):
    out, _ = run(x, freqs, mask, wq, wk, wv, wo)
    return out


# revision 7
# speedup vs baseline: 1.2565x; 1.1901x over previous
"""Trainium2 Bass kernel for causal self-attention with RoPE.

Shapes: x (2, 2048, 2048), 16 heads x 128 head_dim.
Sharding: 8 cores = 2 batch x 4 head-groups (4 heads per core).
Each core computes q/k/v projections for its heads, RoPE, causal-masked
softmax attention, and a partial output projection (its head columns of
wo); the host sums the 4 partials per batch element.

Layout strategy (per core):
  - q,k built in transposed layout (head_dim on partitions, t free) so
    RoPE and the score matmuls need no on-device transposes.  The host
    permutes wq/wk columns so RoPE's even/odd pairs become the two
    partition halves, and pre-scales wq by 1/sqrt(head_dim).
  - scores computed as s^T (keys x q) per 256-query group; softmax skips
    the max-subtraction (scores are O(1) by construction); row sums via
    an ones-vector matmul; normalization folded into the PSUM eviction.
  - v computed directly in (t x e) layout by using x^T as the stationary
    operand, so the p@v matmul needs no transposes anywhere.
  - fully-masked key blocks are skipped (host inspects the mask), which
    halves the attention work for the causal mask; deduplicated mask
    tiles are added only where a block is partially masked.
  - matmuls run in float32r (tf32) which streams at full rate for moving
    dims >= 256.
  - weights/mask stream on the scalar-engine DMA queue, x^T/tables/output
    on the sync-engine queue, so activations never queue behind weights.
"""

import sys
from contextlib import ExitStack

if "/opt/trn_rl_repo" not in sys.path:
    sys.path.insert(0, "/opt/trn_rl_repo")

import numpy as np

import concourse.bacc as bacc
import concourse.mybir as mybir
import concourse.tile as tile
from concourse.bass_utils import run_bass_kernel_spmd

B, T, D, NH, HD = 2, 2048, 2048, 16, 128
HPC = 4              # heads per core
PAIR = 256           # queries per group
NPAIR = T // PAIR    # 8
NCHUNK = T // HD     # 16 key chunks of 128
NSLICE = T // PAIR   # 8 t-slices for projections
F32R = mybir.dt.float32r
F32 = mybir.dt.float32
MASK_PRELOAD_MAX = 24


def _mask_structure(mask):
    """Classify each (query-group, key-chunk) block of the additive mask.

    Returns (statuses, maskt): statuses[j] is a list of
    (chunk, mask_tile_index_or_minus1) for blocks that must be computed;
    maskt is the packed (128, nmask, 256) array of deduplicated
    transposed mask tiles for partially-masked blocks.
    """
    statuses = []
    tiles = {}
    tile_list = []
    for j in range(NPAIR):
        q = slice(j * PAIR, (j + 1) * PAIR)
        lst = []
        for c in range(NCHUNK):
            k = slice(c * HD, (c + 1) * HD)
            sub = mask[q, k]
            if np.all(sub <= -1e8):
                continue
            if np.all(sub == 0.0):
                lst.append((c, -1))
            else:
                key = sub.tobytes()
                mi = tiles.get(key)
                if mi is None:
                    mi = len(tile_list)
                    tiles[key] = mi
                    tile_list.append(np.ascontiguousarray(sub.T))
                lst.append((c, mi))
        assert lst, f"query group {j} has every key block masked"
        statuses.append(lst)
    nmask = max(1, len(tile_list))
    maskt = np.zeros((HD, nmask, PAIR), np.float32)
    for i, t in enumerate(tile_list):
        maskt[:, i, :] = t
    return statuses, maskt


def _build_program(statuses, nmask):
    nc = bacc.Bacc(None, target_bir_lowering=False)

    xt_d = nc.dram_tensor("xt", [D, T], F32R, kind="ExternalInput")
    wq_d = nc.dram_tensor("wqt", [D, HPC * HD], F32R, kind="ExternalInput")
    wk_d = nc.dram_tensor("wkt", [D, HPC * HD], F32R, kind="ExternalInput")
    wv_d = nc.dram_tensor("wvt", [D, HPC * HD], F32R, kind="ExternalInput")
    wo_d = nc.dram_tensor("wot", [HPC * HD, D], F32R, kind="ExternalInput")
    cs_d = nc.dram_tensor("cs", [HD, 2, T], F32, kind="ExternalInput")
    mk_d = nc.dram_tensor("maskt", [HD, nmask, PAIR], F32, kind="ExternalInput")
    ones_d = nc.dram_tensor("ones_col", [HD, 1], F32R, kind="ExternalInput")
    onesr_d = nc.dram_tensor("ones_row", [1, HD], F32R, kind="ExternalInput")
    out_d = nc.dram_tensor("out", [T, D], F32, kind="ExternalOutput")

    xt_ap = xt_d.ap().rearrange("(k p) t -> p k t", p=HD)
    wq_ap = wq_d.ap().rearrange("(k p) e -> p k e", p=HD)
    wk_ap = wk_d.ap().rearrange("(k p) e -> p k e", p=HD)
    wv_ap = wv_d.ap().rearrange("(k p) e -> p k e", p=HD)
    wo_ap = wo_d.ap().rearrange("(h p) e -> p h e", p=HD)
    EXP = mybir.ActivationFunctionType.Exp
    preload_mask = nmask <= MASK_PRELOAD_MAX

    with tile.TileContext(nc) as tc, ExitStack() as top:
        constp = top.enter_context(tc.tile_pool(name="const", bufs=1))
        ones_sb = constp.tile([HD, 1], F32R)
        onesr_sb = constp.tile([1, HD], F32R)
        nc.scalar.dma_start(ones_sb[:], ones_d[:])
        nc.scalar.dma_start(onesr_sb[:], onesr_d[:])

        qkp = top.enter_context(tc.tile_pool(name="qkp", bufs=1))
        # q heads at [:, h, :], k heads at [:, 4+h, :]
        qk_sb = qkp.tile([HD, 2 * HPC, T], F32R)

        # wv + xt pools span the q/k pass (prefetch) and the v pass
        with ExitStack() as vph:
            wvp = vph.enter_context(tc.tile_pool(name="wvp", side="right", bufs=1))
            wv_sb = wvp.tile([HD, NCHUNK, HPC * HD], F32R)
            xtp = vph.enter_context(tc.tile_pool(name="xtp", side="right", bufs=2))

            # ---- combined q/k projection pass (+ fused RoPE) ----
            with ExitStack() as ph:
                wp = ph.enter_context(tc.tile_pool(name="wp", side="right", bufs=1))
                csp = ph.enter_context(tc.tile_pool(name="csp", side="right", bufs=2))
                ropep = ph.enter_context(tc.tile_pool(name="ropep", side="right", bufs=2))
                pps = ph.enter_context(tc.tile_pool(name="pps", bufs=4, space="PSUM"))
                wqk_sb = wp.tile([HD, 2, NCHUNK, HPC * HD], F32R)
                # weights on the scalar queue, split per k-chunk so the
                # first matmuls start as soon as chunk 0 lands
                for k in range(NCHUNK):
                    nc.scalar.dma_start(wqk_sb[:, 0, k, :], wq_ap[:, k, :])
                for k in range(NCHUNK):
                    nc.scalar.dma_start(wqk_sb[:, 1, k, :], wk_ap[:, k, :])
                for k in range(NCHUNK):  # prefetch wv for the next pass
                    nc.scalar.dma_start(wv_sb[:, k, :], wv_ap[:, k, :])
                for ns in range(NSLICE):
                    tsl = slice(ns * PAIR, (ns + 1) * PAIR)
                    xt = xtp.tile([HD, NCHUNK, PAIR], F32R, tag="xt")
                    nc.sync.dma_start(xt[:], xt_ap[:, :, tsl])
                    cs_sl = csp.tile([HD, 2, PAIR], F32, tag="cs")
                    nc.sync.dma_start(cs_sl[:], cs_d[:, :, tsl])
                    for wsel in range(2):
                        for h in range(HPC):
                            ps = pps.tile([HD, PAIR], F32, tag="ps")
                            hs = slice(h * HD, (h + 1) * HD)
                            for k in range(NCHUNK):
                                nc.tensor.matmul(
                                    ps[:],
                                    wqk_sb[:, wsel, k, hs],
                                    xt[:, k, :],
                                    start=(k == 0),
                                    stop=(k == NCHUNK - 1),
                                )
                            # RoPE: dst = raw*C + swap(raw)*S, swap folded
                            # into the second multiply's partition offsets
                            dst = qk_sb[:, wsel * HPC + h, tsl]
                            tb = ropep.tile([HD, PAIR], F32R, tag="tb")
                            nc.vector.tensor_mul(dst, ps[:], cs_sl[:, 0, :])
                            nc.vector.tensor_mul(
                                tb[0:64, :], ps[64:128, :], cs_sl[0:64, 1, :]
                            )
                            nc.vector.tensor_mul(
                                tb[64:128, :], ps[0:64, :], cs_sl[64:128, 1, :]
                            )
                            nc.vector.tensor_add(dst, dst, tb[:])

            # ---- v projection (normal layout, x^T stationary) ----
            vap = top.enter_context(tc.tile_pool(name="vap", bufs=1))
            v_all = vap.tile([HD, NCHUNK, HPC * HD], F32R)
            with ExitStack() as ph:
                vps = ph.enter_context(tc.tile_pool(name="vps", bufs=4, space="PSUM"))
                for ns in reversed(range(NSLICE)):
                    tsl = slice(ns * PAIR, (ns + 1) * PAIR)
                    xt = xtp.tile([HD, NCHUNK, PAIR], F32R, tag="xt")
                    nc.sync.dma_start(xt[:], xt_ap[:, :, tsl])
                    for tc2 in range(2):
                        ps = vps.tile([HD, HPC * HD], F32, tag="vps")
                        for k in range(NCHUNK):
                            nc.tensor.matmul(
                                ps[:],
                                xt[:, k, tc2 * HD:(tc2 + 1) * HD],
                                wv_sb[:, k, :],
                                start=(k == 0),
                                stop=(k == NCHUNK - 1),
                            )
                        nc.scalar.copy(v_all[:, ns * 2 + tc2, :], ps[:])

        # ---- attention ----
        ctxp = top.enter_context(tc.tile_pool(name="ctxp", bufs=1))
        ctx_sb = ctxp.tile([HD, HPC, T], F32R)
        wop = top.enter_context(tc.tile_pool(name="wop", bufs=1))
        wo_sb = wop.tile([HD, HPC, D], F32R)
        with ExitStack() as ph:
            ptp = ph.enter_context(tc.tile_pool(name="ptp", side="right", bufs=2))
            mkp = ph.enter_context(tc.tile_pool(name="mkp", side="right", bufs=4))
            lrp = ph.enter_context(tc.tile_pool(name="lrp", side="right", bufs=2))
            rbp = ph.enter_context(tc.tile_pool(name="rbp", side="right", bufs=2))
            sps = ph.enter_context(tc.tile_pool(name="sps", bufs=2, space="PSUM"))
            ops = ph.enter_context(tc.tile_pool(name="ops", bufs=2, space="PSUM"))
            lps = ph.enter_context(tc.tile_pool(name="lps", bufs=2, space="PSUM"))

            mk_sb = None
            if preload_mask:
                mk_sb = mkp.tile([HD, nmask, PAIR], F32)
                nc.scalar.dma_start(mk_sb[:], mk_d[:])
            for h in range(HPC):  # prefetch wo for the final pass
                nc.scalar.dma_start(wo_sb[:, h, :], wo_ap[:, h, :])

            def mask_tile(mi):
                if preload_mask:
                    return mk_sb[:, mi, :]
                mt = mkp.tile([HD, PAIR], F32, tag="mk")
                nc.scalar.dma_start(mt[:], mk_d[:, mi, :])
                return mt[:]

            def finalize(st):
                # entirely off the tensor engine: DVE recip -> GpSimd
                # partition broadcast -> DVE multiply into ctx
                lr = lrp.tile([1, PAIR], F32, tag="lr")
                nc.vector.reciprocal(lr[:], st["l_ps"][:])
                rb_sb = rbp.tile([HD, PAIR], F32, tag="rb")
                nc.gpsimd.partition_broadcast(rb_sb[:], lr[:])
                nc.vector.tensor_mul(
                    ctx_sb[:, st["h"], st["qsl"]], st["o_ps"][:], rb_sb[:]
                )

            def emit_ol(dq):
                # deferred p@v and row-sum matmuls for an exp'd quad
                pi, quad, st = dq
                h = st["h"]
                for t, (c, mi) in enumerate(quad):
                    nc.tensor.matmul(
                        st["o_ps"][:],
                        v_all[:, c, h * HD:(h + 1) * HD],
                        st["pt"][:, pi + t, :],
                        start=(st["oi"] == 0),
                        stop=(st["oi"] == st["n"] - 1),
                    )
                    st["oi"] += 1
                for t, (c, mi) in enumerate(quad):
                    nc.tensor.matmul(
                        st["l_ps"][:],
                        ones_sb[:],
                        st["pt"][:, pi + t, :],
                        start=(st["li"] == 0),
                        stop=(st["li"] == st["n"] - 1),
                    )
                    st["li"] += 1
                return st["li"] == st["n"]  # group's accumulation complete

            pending_ol = None
            pending_fin = None
            for j in reversed(range(NPAIR)):
                qsl = slice(j * PAIR, (j + 1) * PAIR)
                chunks = statuses[j]
                n = len(chunks)
                quads = [chunks[ii:ii + 4] for ii in range(0, n, 4)]
                for h in range(HPC):
                    o_ps = ops.tile([HD, PAIR], F32, tag="o")
                    l_ps = lps.tile([1, PAIR], F32, tag="l")
                    pt = ptp.tile([HD, NCHUNK, PAIR], F32R, tag="pt")
                    st = {"o_ps": o_ps, "l_ps": l_ps, "pt": pt, "h": h,
                          "qsl": qsl, "n": n, "oi": 0, "li": 0}
                    for qi, quad in enumerate(quads):
                        w = len(quad)
                        s_ps = sps.tile([HD, 4, PAIR], F32, tag="s")
                        for t, (c, mi) in enumerate(quad):
                            nc.tensor.matmul(
                                s_ps[:, t, :],
                                qk_sb[:, HPC + h, c * HD:(c + 1) * HD],
                                qk_sb[:, h, qsl],
                                start=True,
                                stop=True,
                            )
                        t = 0
                        while t < w:  # additive mask, merging adjacent tiles
                            c, mi = quad[t]
                            if mi < 0:
                                t += 1
                            elif (t + 1 < w and quad[t + 1][1] == mi + 1
                                  and preload_mask):
                                nc.vector.tensor_add(
                                    s_ps[:, t:t + 2, :], s_ps[:, t:t + 2, :],
                                    mk_sb[:, mi:mi + 2, :],
                                )
                                t += 2
                            else:
                                nc.vector.tensor_add(
                                    s_ps[:, t, :], s_ps[:, t, :], mask_tile(mi)
                                )
                                t += 1
                        nc.scalar.activation(
                            pt[:, qi * 4:qi * 4 + w, :], s_ps[:, 0:w, :], EXP
                        )
                        if pending_ol is not None:
                            if emit_ol(pending_ol):
                                pending_fin = pending_ol[2]
                            pending_ol = None
                        if pending_fin is not None and pending_fin is not st:
                            finalize(pending_fin)
                            pending_fin = None
                        pending_ol = (qi * 4, quad, st)
            if pending_ol is not None:
                if emit_ol(pending_ol):
                    pending_fin = pending_ol[2]
            if pending_fin is not None:
                finalize(pending_fin)

        # ---- output projection ----
        with ExitStack() as ph:
            evp = ph.enter_context(tc.tile_pool(name="evp", side="right", bufs=4))
            wops = ph.enter_context(tc.tile_pool(name="wops", bufs=4, space="PSUM"))
            for tck in range(NCHUNK):
                tsl = slice(tck * HD, (tck + 1) * HD)
                for es in range(4):
                    esl = slice(es * 512, (es + 1) * 512)
                    ps = wops.tile([HD, 512], F32, tag="wo")
                    for h in range(HPC):
                        nc.tensor.matmul(
                            ps[:],
                            ctx_sb[:, h, tsl],
                            wo_sb[:, h, esl],
                            start=(h == 0),
                            stop=(h == HPC - 1),
                        )
                    ev = evp.tile([HD, 512], F32, tag="ev")
                    nc.scalar.copy(ev[:], ps[:])
                    nc.sync.dma_start(out_d[tsl, esl], ev[:])
    nc.compile()
    return nc


_PERM = np.concatenate(
    [np.concatenate([np.arange(0, HD, 2), np.arange(1, HD, 2)]) + h * HD
     for h in range(HPC)]
)


def prepare(x, freqs, mask, wq, wk, wv, wo):
    """Host-side sharding/prep. Returns (nc, in_maps)."""
    x = np.asarray(x, np.float32)
    freqs = np.asarray(freqs, np.float32)
    mask = np.asarray(mask, np.float32)
    wq, wk, wv, wo = (np.asarray(w, np.float32) for w in (wq, wk, wv, wo))

    statuses, maskt = _mask_structure(mask)
    nc = _build_program(statuses, maskt.shape[1])

    scale = np.float32(1.0 / np.sqrt(HD))
    cos = np.ascontiguousarray(freqs[:, :, 0].T)  # (64, T)
    sin = np.ascontiguousarray(freqs[:, :, 1].T)
    cs = np.empty((HD, 2, T), np.float32)
    cs[0:64, 0, :] = cos
    cs[64:128, 0, :] = cos
    cs[0:64, 1, :] = -sin
    cs[64:128, 1, :] = sin

    ones_col = np.ones((HD, 1), np.float32)
    ones_row = np.ones((1, HD), np.float32)
    xt = [np.ascontiguousarray(x[b].T) for b in range(B)]

    in_maps = []
    for core in range(8):
        b, g = core // 4, core % 4
        cols = slice(g * HPC * HD, (g + 1) * HPC * HD)
        in_maps.append({
            "xt": xt[b],
            "wqt": np.ascontiguousarray((wq.T[:, cols] * scale)[:, _PERM]),
            "wkt": np.ascontiguousarray(wk.T[:, cols][:, _PERM]),
            "wvt": np.ascontiguousarray(wv.T[:, cols]),
            "wot": np.ascontiguousarray(wo.T[cols, :]),
            "cs": cs,
            "maskt": maskt,
            "ones_col": ones_col,
            "ones_row": ones_row,
        })
    return nc, in_maps


def run(x, freqs, mask, wq, wk, wv, wo, **spmd_kwargs):
    nc, in_maps = prepare(x, freqs, mask, wq, wk, wv, wo)
    res = run_bass_kernel_spmd(nc, in_maps, list(range(8)), **spmd_kwargs)
    parts = [res.results[c]["out"] for c in range(8)]
    out = np.stack([
        parts[b * 4] + parts[b * 4 + 1] + parts[b * 4 + 2] + parts[b * 4 + 3]
        for b in range(B)
    ]).astype(np.float32)
    return out, res


def kernel(x, freqs, mask, wq, wk, wv, wo):
    out, _ = run(x, freqs, mask, wq, wk, wv, wo)
    return out
